# revision 6
# baseline (speedup 1.0000x reference)
"""MLA (multi-head latent attention) Bass kernel for Trainium2, 8 NeuronCores.

Sharding: data-parallel over batch (cores 0-3 = batch 0, cores 4-7 = batch 1),
tensor-parallel over heads within each group (4 of 16 heads per core).
Per-core pipeline (all matmuls fp32r at full PE rate):
  1. q_latT/kv_latT shard = Wq/Wkv shard @ x^T          (latent-on-partition layout)
  2. AllGather kv_latT within group -> full latent for V up-projection
  3. v = kv_latT_full^T @ Wvb shard^T                   ([seq, vd] layout)
  4. causal attention per head in [k, q] orientation:
       scoresT = k^T q, exp on ACT, causal mask on diagonal blocks,
       denominators via ones-matmul, out^T accumulated as v^T @ exp,
       divide by denominators via PE outer-product broadcast
  5. AllGather attention outputs within group -> full [hvd, seq]
  6. out^T shard = Wo_d shard @ attn^T  (1792 of 7168 output dims per core)
Host side: pre-transpose/shard inputs, gather+transpose outputs, add bias.
"""

import numpy as np

import concourse.bacc as bacc
import concourse.bass as bass
import concourse.mybir as mybir
import concourse.tile as tile
from concourse.bass_utils import run_bass_kernel_spmd

# Problem constants (nn_MLA_50379966382638)
B, S, D = 2, 2048, 7168
R, H, VD = 1024, 16, 128
QK_HD = R // H            # 64
SCALE = float(np.sqrt(D // H))

N_CORES = 8
TP = 4                    # tensor-parallel ranks per batch group
HPC = H // TP             # 4 heads per core
RS = R // TP              # 256 latent dims per core
VS = HPC * VD             # 512 value dims per core
DS = D // TP              # 1792 output dims per core
GROUPS = [[0, 1, 2, 3], [4, 5, 6, 7]]

DM_CH = D // 128          # 56 d_model chunks
SQ_CH = S // 512          # 4 seq chunks of 512 (moving dim)
KC_CH = S // 128          # 16 seq chunks of 128 (key blocks)
HV_CH = (H * VD) // 128   # 16 hvd chunks
DO_CH = DS // 128         # 14 output-dim chunks per core

F32 = mybir.dt.float32
F32R = mybir.dt.float32r
EXP = mybir.ActivationFunctionType.Exp

TRACE = False             # set True from test harness to capture NTFF profile
_CACHE = {}


def _emit(nc, tc, xT, wqT, wkvT, wvbT, woT, masks, ones, outT):
    ts = bass.ts

    with (
        tc.tile_pool(name="const", bufs=1) as const_pool,
        tc.tile_pool(name="qkv", bufs=1) as qkv_pool,
        tc.tile_pool(name="dram", bufs=1, space="DRAM") as dram_pool,
    ):
        # constants: causal masks for the 4 diagonal offsets + ones vectors
        mask_t = []
        for j in range(4):
            m = const_pool.tile([128, 512], F32R, tag=f"mask{j}", name=f"mask{j}")
            nc.sync.dma_start(m[:], masks[j])
            mask_t.append(m)
        ones_t = const_pool.tile([128, 128], F32R, tag="ones", name="ones_t")
        nc.sync.dma_start(ones_t[:], ones[:])
        ones_col = ones_t[:, 0:1]
        ones_row = ones_t[0:1, :]

        # results of the latent projections, [128 latent, S] per tile
        qlat = [qkv_pool.tile([128, S], F32R, tag=f"qlat{i}", name=f"qlat{i}") for i in range(2)]
        kvlat = [qkv_pool.tile([128, S], F32R, tag=f"kvlat{i}", name=f"kvlat{i}") for i in range(2)]

        # ---- Stage P: latent projections q_latT / kv_latT ----
        with (
            tc.tile_pool(name="wproj", bufs=1) as w_pool,
            tc.tile_pool(name="xs", bufs=3) as x_pool,
            tc.tile_pool(name="pps", bufs=2, space="PSUM") as pps,
        ):
            wq_t, wkv_t = [], []
            for dm in range(DM_CH):
                wq = w_pool.tile([128, RS], F32R, tag=f"wq{dm}", name=f"wq{dm}")
                nc.sync.dma_start(wq[:], wqT[ts(dm, 128), :])
                wq_t.append(wq)
                wkv = w_pool.tile([128, RS], F32R, tag=f"wkv{dm}", name=f"wkv{dm}")
                nc.sync.dma_start(wkv[:], wkvT[ts(dm, 128), :])
                wkv_t.append(wkv)

            for qc in range(SQ_CH):
                accs = [pps.tile([128, 512], F32, tag=f"pacc{i}", name=f"pacc{i}_{qc}") for i in range(4)]
                for dm in range(DM_CH):
                    xt = x_pool.tile([128, 512], F32R, tag="xt", name=f"xt{qc}_{dm}")
                    nc.sync.dma_start(xt[:], xT[ts(dm, 128), ts(qc, 512)])
                    st, sp = dm == 0, dm == DM_CH - 1
                    for i in range(2):
                        nc.tensor.matmul(accs[i][:], wq_t[dm][:, ts(i, 128)],
                                         xt[:], start=st, stop=sp)
                        nc.tensor.matmul(accs[2 + i][:], wkv_t[dm][:, ts(i, 128)],
                                         xt[:], start=st, stop=sp)
                for i in range(2):
                    nc.scalar.copy(qlat[i][:, ts(qc, 512)], accs[i][:])
                    nc.vector.tensor_copy(kvlat[i][:, ts(qc, 512)], accs[2 + i][:])

        # ---- Stage AG1: AllGather kv latent within the batch group ----
        kv_bounce_in = dram_pool.tile([RS, S], F32R, tag="kvbi", name="kvbi")
        kv_bounce_out = dram_pool.tile([R, S], F32R, tag="kvbo", name="kvbo")
        for i in range(2):
            nc.sync.dma_start(kv_bounce_in[ts(i, 128), :], kvlat[i][:])
        nc.gpsimd.collective_compute(
            "AllGather", mybir.AluOpType.bypass, replica_groups=GROUPS,
            ins=[kv_bounce_in[:].opt()], outs=[kv_bounce_out[:].opt()],
        )

        # ---- Stage V: v = kv_lat_full @ Wvb_sh^T, [seq, vd] layout ----
        with tc.tile_pool(name="vsb", bufs=1) as v_pool:
            v_t = [v_pool.tile([128, VS], F32R, tag=f"v{s}", name=f"v{s}") for s in range(KC_CH)]
            with (
                tc.tile_pool(name="kvf", bufs=1) as kvf_pool,
                tc.tile_pool(name="wvb", bufs=1) as wvb_pool,
                tc.tile_pool(name="vps", bufs=2, space="PSUM") as vps,
            ):
                kvf_t, wvb_t = [], []
                for lc in range(R // 128):
                    kf = kvf_pool.tile([128, S], F32R, tag=f"kvf{lc}", name=f"kvf{lc}")
                    nc.sync.dma_start(kf[:], kv_bounce_out[ts(lc, 128), :])
                    kvf_t.append(kf)
                    wv = wvb_pool.tile([128, VS], F32R, tag=f"wvb{lc}", name=f"wvb{lc}")
                    nc.sync.dma_start(wv[:], wvbT[ts(lc, 128), :])
                    wvb_t.append(wv)
                for s in range(KC_CH):
                    acc = vps.tile([128, VS], F32, tag="vacc", name=f"vacc{s}")
                    for lc in range(R // 128):
                        nc.tensor.matmul(acc[:], kvf_t[lc][:, ts(s, 128)], wvb_t[lc][:],
                                         start=(lc == 0), stop=(lc == R // 128 - 1))
                    if s % 2 == 0:
                        nc.scalar.copy(v_t[s][:], acc[:])
                    else:
                        nc.vector.tensor_copy(v_t[s][:], acc[:])

            # ---- Stage A: causal attention per local head ----
            at_bounce_in = dram_pool.tile([VS, S], F32R, tag="atbi", name="atbi")
            at_bounce_out = dram_pool.tile([H * VD, S], F32R, tag="atbo", name="atbo")
            with (
                tc.tile_pool(name="aout", bufs=1) as aout_pool,
                tc.tile_pool(name="exs", bufs=3) as ex_pool,
                tc.tile_pool(name="small", bufs=4) as small_pool,
                tc.tile_pool(name="aps", bufs=2, space="PSUM") as aps,
                tc.tile_pool(name="bps", bufs=1, space="PSUM") as bps,
            ):
                aout = [aout_pool.tile([128, S], F32R, tag=f"ao{h}", name=f"ao{h}") for h in range(HPC)]
                for h in range(HPC):
                    ti, r0 = h // 2, (h % 2) * 64
                    for qc in range(SQ_CH):
                        av = aps.tile([128, 512], F32, tag="av", name=f"av{h}_{qc}")
                        sm = aps.tile([1, 512], F32, tag="sm", name=f"sm{h}_{qc}")
                        nkc = 4 * qc + 4
                        for kc in range(nkc):
                            sc = aps.tile([128, 512], F32, tag="sc", name=f"sc{h}_{qc}_{kc}")
                            nc.tensor.matmul(
                                sc[:],
                                kvlat[ti][r0:r0 + 64, ts(kc, 128)],
                                qlat[ti][r0:r0 + 64, ts(qc, 512)],
                                start=True, stop=True)
                            ex = ex_pool.tile([128, 512], F32R, tag="ex", name=f"ex{h}_{qc}_{kc}")
                            nc.scalar.activation(ex[:], sc[:], EXP, scale=1.0 / SCALE)
                            j = kc - 4 * qc
                            if j >= 0:
                                nc.vector.tensor_mul(ex[:], ex[:], mask_t[j][:])
                            st, sp = kc == 0, kc == nkc - 1
                            nc.tensor.matmul(sm[:], ones_col, ex[:],
                                             start=st, stop=sp)
                            nc.tensor.matmul(av[:], v_t[kc][:, ts(h, 128)], ex[:],
                                             start=st, stop=sp)
                        rc = small_pool.tile([1, 512], F32R, tag="rc", name=f"rc{h}_{qc}")
                        with nc.allow_low_precision(reason="f32r is bit-identical to f32"):
                            nc.vector.reciprocal(rc[:], sm[:])
                        bc = bps.tile([128, 512], F32, tag="bc", name=f"bc{h}_{qc}")
                        nc.tensor.matmul(bc[:], ones_row, rc[:],
                                         start=True, stop=True)
                        bcs = small_pool.tile([128, 512], F32R, tag="bcs", name=f"bcs{h}_{qc}")
                        nc.scalar.copy(bcs[:], bc[:])
                        nc.vector.tensor_mul(aout[h][:, ts(qc, 512)], av[:], bcs[:])

                # ---- Stage AG2: AllGather attention outputs ----
                for h in range(HPC):
                    nc.sync.dma_start(at_bounce_in[ts(h, 128), :], aout[h][:])
            nc.gpsimd.collective_compute(
                "AllGather", mybir.AluOpType.bypass, replica_groups=GROUPS,
                ins=[at_bounce_in[:].opt()], outs=[at_bounce_out[:].opt()],
            )

        # ---- Stage WO: out^T shard = Wo_d @ attn^T ----
        with (
            tc.tile_pool(name="atf", bufs=1) as atf_pool,
            tc.tile_pool(name="wo", bufs=2) as wo_pool,
            tc.tile_pool(name="osb", bufs=4) as o_pool,
            tc.tile_pool(name="ops", bufs=4, space="PSUM") as ops,
        ):
            atf_t = []
            for hv in range(HV_CH):
                af = atf_pool.tile([128, S], F32R, tag=f"atf{hv}", name=f"atf{hv}")
                nc.sync.dma_start(af[:], at_bounce_out[ts(hv, 128), :])
                atf_t.append(af)
            for d in range(DO_CH):
                wod = []
                for hv in range(HV_CH):
                    w = wo_pool.tile([128, 128], F32R, tag=f"wo{hv}", name=f"wo{d}_{hv}")
                    nc.sync.dma_start(w[:], woT[ts(hv, 128), ts(d, 128)])
                    wod.append(w)
                for qc in range(SQ_CH):
                    acc = ops.tile([128, 512], F32, tag="oacc", name=f"oacc{d}_{qc}")
                    for hv in range(HV_CH):
                        nc.tensor.matmul(acc[:], wod[hv][:], atf_t[hv][:, ts(qc, 512)],
                                         start=(hv == 0), stop=(hv == HV_CH - 1))
                    ot = o_pool.tile([128, 512], F32, tag="ot", name=f"ot{d}_{qc}")
                    if qc % 2 == 0:
                        nc.scalar.copy(ot[:], acc[:])
                    else:
                        nc.vector.tensor_copy(ot[:], acc[:])
                    nc.sync.dma_start(outT[ts(d, 128), ts(qc, 512)], ot[:])


def _build():
    if "nc" in _CACHE:
        return _CACHE["nc"]
    nc = bacc.Bacc("TRN2", target_bir_lowering=False, debug=False,
                   num_devices=N_CORES)
    xT = nc.dram_tensor("xT", [D, S], F32R, kind="ExternalInput").ap()
    wqT = nc.dram_tensor("wqT", [D, RS], F32R, kind="ExternalInput").ap()
    wkvT = nc.dram_tensor("wkvT", [D, RS], F32R, kind="ExternalInput").ap()
    wvbT = nc.dram_tensor("wvbT", [R, VS], F32R, kind="ExternalInput").ap()
    woT = nc.dram_tensor("woT", [H * VD, DS], F32R, kind="ExternalInput").ap()
    masks = nc.dram_tensor("masks", [4, 128, 512], F32R, kind="ExternalInput").ap()
    ones = nc.dram_tensor("ones", [128, 128], F32R, kind="ExternalInput").ap()
    outT = nc.dram_tensor("outT", [DS, S], F32, kind="ExternalOutput").ap()
    with tile.TileContext(nc) as tc:
        _emit(nc, tc, xT, wqT, wkvT, wvbT, woT, masks, ones, outT)
    nc.compile()
    _CACHE["nc"] = nc
    return nc


def _host_masks():
    p = np.arange(128, dtype=np.float32)[:, None]
    f = np.arange(512, dtype=np.float32)[None, :]
    return np.stack([(p + 128 * j <= f).astype(np.float32) for j in range(4)])


def _in_maps(inputs):
    x = np.asarray(inputs["x"], dtype=np.float32)
    Wq = np.asarray(inputs["Wq"], np.float32)
    Wkv = np.asarray(inputs["Wkv"], np.float32)
    Wvb = np.asarray(inputs["Wvb"], np.float32)
    Wo = np.asarray(inputs["Wo"], np.float32)
    masks = _host_masks()
    xTs = [np.ascontiguousarray(x[g].T) for g in range(B)]
    in_maps = []
    for c in range(N_CORES):
        g, t = c // TP, c % TP
        in_maps.append({
            "xT": xTs[g],
            "wqT": np.ascontiguousarray(Wq[t * RS:(t + 1) * RS, :].T),
            "wkvT": np.ascontiguousarray(Wkv[t * RS:(t + 1) * RS, :].T),
            "wvbT": np.ascontiguousarray(Wvb[t * VS:(t + 1) * VS, :].T),
            "woT": np.ascontiguousarray(Wo[t * DS:(t + 1) * DS, :].T),
            "masks": masks,
            "ones": np.ones((128, 128), np.float32),
        })
    return in_maps


def _assemble(results, bo):
    bo = np.asarray(bo, np.float32)
    out = np.empty((B, S, D), dtype=np.float32)
    for c in range(N_CORES):
        g, t = c // TP, c % TP
        out[g, :, t * DS:(t + 1) * DS] = results[c]["outT"].T
    if bo.any():
        out += bo
    return out


def kernel(x, Wq, Wkv, Wvb, Wo, bo):
    nc = _build()
    in_maps = _in_maps(dict(x=x, Wq=Wq, Wkv=Wkv, Wvb=Wvb, Wo=Wo))
    res = run_bass_kernel_spmd(nc, in_maps, core_ids=list(range(N_CORES)))
    return _assemble(res.results, bo)
